# revision 46
# baseline (speedup 1.0000x reference)
"""Trainium2 Bass kernel for NormalAttention (embedded gaussian, non-local block).

Reference computation per batch sample b (B=8, C=256, Ck=64, N=48*48=2304):
    q = Wq @ x + bq            (64, N)
    k = Wk @ x + bk            (64, N)
    e[i,j] = q[:,i] . k[:,j]   (N, N)
    E = exp(e);  E[i,j] /= sum_j E[i,j]
    v = Wv @ x + bv            (256, N)
    att[c,j] = sum_i v[c,i] * E[i,j]
    out = Wg @ att + bg        (256, N)

Sharding: pure data parallel, one batch sample per NeuronCore (8 cores).

Per-core kernel structure (cost-model timed at ~82.9us/core):
  - all matmuls in bf16 (1 cycle/row on PE vs 4 for fp32); rel err ~4.5e-3.
  - energy rows computed one 128-row i-chunk at a time into ping-pong PSUM
    tiles ((128,1280)+(128,1024) = 5 banks); exp on ACT engine straight out
    of PSUM into resident bf16 expA/expB SBUF tensors. Steady-state phase 1
    is ACT-paced at ~2.48us/chunk (exp 1.92us + per-op init + accum read).
  - row sums: ACT accum_out on the 1280-half + DVE tensor_reduce on the
    1024-half (expE split into two tiles so the reduce never falsely
    serializes against the next exp's SBUF write); row normalization is
    folded into V^T (vt[i,:] *= 1/s[i]) on DVE.
  - the gamma 1x1 conv is folded into the V projection on the host
    (W_comb = (Wg@Wv)^T, bvg = Wg@bv), so pass 2 (out = vg^T.T @ expE)
    directly produces final outputs: PSUM-accumulate over the 18 i-chunks,
    add gamma_bias on DVE, store. 3 of the 10 (oc, j-tile) accumulator
    groups stream into phase 1's idle PE slots (PSUM-bank limited); the
    remaining groups run in phase 2, the first two out of the freed energy
    banks, smallest j-tiles last to minimize the tail.
  - head: PE warmup matmuls + a 1-element exp (prefetches the ACT
    function table) under the input DMAs; x arrives in 3 column pieces
    ordered to unblock the Q/K projection chain (k-bias on DVE and q-bias
    on ACT run as parallel PSUM->SBUF chains).
"""

import os
import sys

sys.path.insert(0, "/opt/trn_rl_repo")

# NTFF tracing is unavailable through this container's axon client; make sure
# a stray BASS_TRACE in the environment can't break the execution path.
os.environ["BASS_NEVER_TRACE"] = "1"

# This kernel executes through the axon-proxied PJRT backend. If the caller's
# environment pinned jax to CPU (common for reference-side runs), drop the pin
# before jax initializes so the TRN2 devices stay discoverable.
_jp = os.environ.get("JAX_PLATFORMS")
if _jp and "axon" not in _jp and "jax" not in sys.modules:
    os.environ.pop("JAX_PLATFORMS", None)

import numpy as np
import ml_dtypes

import concourse.bass as bass
import concourse.mybir as mybir
import concourse.tile as tile
from concourse import bacc
from concourse.bass_utils import run_bass_kernel_spmd

B, C, CK, H, W = 8, 256, 64, 48, 48
N = H * W            # 2304
P = 128
NI = N // P          # 18 i-chunks
NCORES = 8

BF16 = mybir.dt.bfloat16
F32 = mybir.dt.float32
AF = mybir.ActivationFunctionType
ALU = mybir.AluOpType
AX = mybir.AxisListType

# energy ping-pong PSUM split: (128,1280)=3 banks + (128,1024)=2 banks.
# expE is likewise stored as two SBUF tiles (A: j<1280, B: j>=1280) so the
# DVE row-sum reduce of half B never falsely serializes with the next exp.
EA, EB = 1280, 1024
E_SPLITS = [
    (0, EA, "engA", [(0, 512), (512, 512), (1024, 256)]),
    (EA, EB, "engB", [(0, 512), (512, 512)]),
]
# pass-2 j-tiling must nest inside the A/B split
J_TILES = [(0, 512), (512, 512), (1024, 256), (1280, 512), (1792, 512)]
# pass-2 groups (ch, j0), jw=512, streamed into phase 1 (all within half A)
STREAM_GROUPS = [(0, 0), (1, 0), (0, 512)]

N_WARM = 7           # PE warmup matmuls issued under the input DMAs


def _build_nc():
    nc = bacc.Bacc("TRN2", target_bir_lowering=False, debug=False,
                   num_devices=NCORES)

    x_d = nc.dram_tensor("x", [2, P, N], BF16, kind="ExternalInput")
    wqk_d = nc.dram_tensor("wqk", [P, 2 * P], BF16, kind="ExternalInput")
    wrest_d = nc.dram_tensor("wrest", [P, 2 * C], BF16, kind="ExternalInput")
    fblob_d = nc.dram_tensor("fblob", [P, C + 4], F32, kind="ExternalInput")
    out_d = nc.dram_tensor("out", [2, P, N], F32, kind="ExternalOutput")
    warm_d = nc.dram_tensor("warm", [P, 1], F32, kind="ExternalOutput")

    with tile.TileContext(nc) as tc:
        with (
            tc.tile_pool(name="consts", bufs=1) as consts,
            tc.tile_pool(name="big", bufs=1) as big,
            tc.tile_pool(name="work", bufs=6) as work,
            tc.tile_pool(name="ps_big", bufs=1, space="PSUM") as ps_big,
            tc.tile_pool(name="ps_sm", bufs=2, space="PSUM") as ps_sm,
            tc.tile_pool(name="ps_st", bufs=1, space="PSUM") as ps_st,
        ):
            # ---------------- PE warmup under the input DMAs ----------------
            dummy = consts.tile([P, 512], BF16)
            nc.gpsimd.memset(dummy[:], 0)
            warm_sb = consts.tile([P, 1], F32)
            # 1-element exp: forces the implicit ACT_TABLE_LOAD (~1.3us) to
            # run at t~0 under the DMAs instead of gating the first q-bias
            nc.scalar.activation(warm_sb[0:1, 0:1], dummy[0:1, 0:1], AF.Exp)
            psd = ps_sm.tile([P, 512], F32, tag="sm")
            for w in range(N_WARM):
                nc.tensor.matmul(psd[:], dummy[:, :P], dummy[:],
                                 start=(w == 0), stop=(w == N_WARM - 1))
            nc.vector.tensor_copy(warm_sb, psd[:, 0:1])
            nc.sync.dma_start(warm_d[:], warm_sb)

            # ---------------- inputs ----------------
            # order: biases + wqk (tiny) first, then x in j-halves so the
            # first Q/K projection tiles can start before x fully lands
            wqk = consts.tile([P, 2 * P], BF16)
            nc.sync.dma_start(wqk[:], wqk_d[:])
            xt = big.tile([P, 2, N], BF16)
            fblob = consts.tile([P, C + 4], F32)
            x_r = x_d[:].rearrange("c p n -> p c n")
            nc.sync.dma_start(xt[:, :, 0:512], x_r[:, :, 0:512])
            nc.sync.dma_start(fblob, fblob_d[:])
            nc.sync.dma_start(xt[:, :, 512:1280], x_r[:, :, 512:1280])
            nc.sync.dma_start(xt[:, :, 1280:N], x_r[:, :, 1280:N])
            wrest = consts.tile([P, 2 * C], BF16)
            nc.sync.dma_start(wrest[:], wrest_d[:])

            # combined projection weight W_comb = (Wg @ Wv)^T: the gamma
            # 1x1 conv is folded into the V projection on the host, so
            # pass-2 outputs are final (no gamma matmuls, no att copies)
            def wv(c):
                return wrest[:, c * C:(c + 1) * C]

            qb = fblob[0:CK, 0:1]
            kb = fblob[0:CK, 1:2]
            vb = fblob[:, 4:C + 4]
            gbias = fblob[:, 2:4]

            # ---------------- Q / K projections ----------------
            q_t = big.tile([CK, N], BF16)
            k_t = big.tile([CK, N], BF16)

            # k-bias lands on DVE, q-bias on ACT (parallel PSUM->SBUF chains;
            # the energy matmuls are gated mostly on k_t). The q-bias for a
            # j-tile can be deferred (only q_t[:, k*128:(k+1)*128] gates
            # chunk k's energy row).
            def psk_mms(j0, jw, pool=None, tag="sm"):
                psk = (pool or ps_sm).tile([P, 512], F32, tag=tag, name="psk")
                for c in range(2):
                    nc.tensor.matmul(psk[:CK, :jw],
                                     wqk[:, c * P + CK:(c + 1) * P],
                                     xt[:, c, j0:j0 + jw],
                                     start=(c == 0), stop=(c == 1))
                nc.vector.tensor_scalar_add(k_t[:, j0:j0 + jw], psk[:CK, :jw], kb)

            def psq_mms(j0, jw, on_act=False):
                psq = ps_sm.tile([P, 512], F32, tag="sm", name="psq")
                for c in range(2):
                    nc.tensor.matmul(psq[:CK, :jw], wqk[:, c * P:c * P + CK],
                                     xt[:, c, j0:j0 + jw],
                                     start=(c == 0), stop=(c == 1))
                if on_act:
                    # ACT is idle during the head; q_t[:, :128] gates exp(0)
                    nc.scalar.activation(q_t[:, j0:j0 + jw], psq[:CK, :jw],
                                         AF.Identity, bias=qb)
                else:
                    nc.vector.tensor_scalar_add(q_t[:, j0:j0 + jw],
                                                psq[:CK, :jw], qb)

            # shared big SBUF tensors
            vt = big.tile([P, NI, C], BF16)       # V^T, later scaled by 1/s
            expA = big.tile([P, NI, EA], BF16)
            expB = big.tile([P, NI, EB], BF16)
            s_half = big.tile([P, NI, 2], F32)
            invs = big.tile([P, NI], F32)

            eps_of = {}

            def emit_energy(kk, part):
                (base, width, tag, subs) = E_SPLITS[part]
                eps = ps_big.tile([P, width], F32, tag=tag, name=f"eps{part}")
                for (o0, ow) in subs:
                    nc.tensor.matmul(
                        eps[:, o0:o0 + ow],
                        q_t[:, kk * P:(kk + 1) * P],
                        k_t[:, base + o0:base + o0 + ow],
                        start=True, stop=True)
                eps_of.setdefault(kk, [None, None])[part] = eps

            # Q/K for j < 1280, then E(0) half A right away; rest of Q/K,
            # then E(0) half B -- gets the first exp started ASAP. The
            # q-biases of the last two j-tiles are deferred into the loop
            # (not needed until energy chunk 10) to keep ACT clear.
            # critical chain to exp(0): k_t[:, 0:1280] + q_t[:, 0:128];
            # run both bias chains (DVE for k, ACT for q) concurrently,
            # borrowing the stream slot (idle until chunk 1) for psk j1
            psk_mms(*J_TILES[0])
            psq_mms(*J_TILES[0], on_act=True)
            psk_mms(*J_TILES[1], pool=ps_st, tag="st")
            psk_mms(*J_TILES[2])
            emit_energy(0, 0)
            psq_mms(*J_TILES[1])
            psq_mms(*J_TILES[2])
            for (j0, jw) in J_TILES[3:]:
                psk_mms(j0, jw)
            emit_energy(0, 1)
            psq_mms(*J_TILES[3])
            psq_mms(*J_TILES[4])

            # ---------------- V^T projection ----------------
            # emitted after E(0); demoted in scheduler priority so the
            # chains only fill genuine PE idle slots instead of front-
            # running the energy matmuls of the first pass-1 chunks
            with tc.high_priority(offset=-100000):
                for i in range(NI):
                    psv = ps_sm.tile([P, 512], F32, tag="sm")
                    for c in range(2):
                        nc.tensor.matmul(psv[:, :C],
                                         xt[:, c, i * P:(i + 1) * P],
                                         wv(c), start=(c == 0), stop=(c == 1))
                    nc.vector.tensor_tensor(vt[:, i], psv[:, :C], vb, ALU.add)

            # ---------------- pass 1 pipeline ----------------
            st_tiles = []
            for k in range(NI):
                # exp of both energy halves; row-sum of half A via ACT accum
                nc.scalar.activation(
                    out=expA[:, k, :], in_=eps_of[k][0][:],
                    func=AF.Exp, accum_out=s_half[:, k, 0:1])
                nc.scalar.activation(
                    out=expB[:, k, :], in_=eps_of[k][1][:],
                    func=AF.Exp)
                # row-sum of half B on DVE; s = sA + sB; invs = 1/s
                nc.vector.tensor_reduce(
                    s_half[:, k, 1:2], expB[:, k, :],
                    axis=AX.X, op=ALU.add)
                nc.vector.tensor_tensor(invs[:, k:k + 1], s_half[:, k, 0:1],
                                        s_half[:, k, 1:2], ALU.add)
                nc.vector.reciprocal(invs[:, k:k + 1], invs[:, k:k + 1])
                nc.vector.tensor_scalar_mul(vt[:, k], vt[:, k],
                                            invs[:, k:k + 1])
                # PE order per chunk: E(k+1)A (unblocks the next exp ASAP),
                # then streamed pass-2 matmuls for chunk k-1 covering the
                # wait for exp(k)B's PSUM read, then E(k+1)B.
                if k + 1 < NI:
                    emit_energy(k + 1, 0)
                if k >= 1:
                    kk = k - 1
                    if kk == 0:
                        st_tiles = [
                            (ps_st if gi == 0 else ps_sm).tile(
                                [P, 512], F32,
                                tag=("st" if gi == 0 else "sm"),
                                name=f"stream_{gi}")
                            for gi in range(len(STREAM_GROUPS))
                        ]
                    for gi, (ch, j0) in enumerate(STREAM_GROUPS):
                        nc.tensor.matmul(
                            st_tiles[gi][:],
                            vt[:, kk, ch * P:(ch + 1) * P],
                            expA[:, kk, j0:j0 + 512],
                            start=(kk == 0), stop=False)
                if k + 1 < NI:
                    emit_energy(k + 1, 1)

            # ---------------- pass 2 ----------------
            def exp_slice(i, j0, jw):
                if j0 + jw <= EA:
                    return expA[:, i, j0:j0 + jw]
                return expB[:, i, j0 - EA:j0 - EA + jw]

            def emit_out(oc, j0, jw, psum_ap):
                ot = work.tile([P, 512], F32, tag="out")
                nc.vector.tensor_scalar_add(ot[:, :jw], psum_ap,
                                            gbias[:, oc:oc + 1])
                nc.sync.dma_start(out_d[oc, :, j0:j0 + jw], ot[:, :jw])

            def full_group(oc, j0, jw, pool=None, tag="sm"):
                aps = (pool or ps_sm).tile([P, 512], F32, tag=tag, name="aps")
                for i in range(NI):
                    nc.tensor.matmul(
                        aps[:, :jw],
                        vt[:, i, oc * P:(oc + 1) * P],
                        exp_slice(i, j0, jw),
                        start=(i == 0), stop=(i == NI - 1))
                emit_out(oc, j0, jw, aps[:, :jw])

            # First two groups run out of the (now free) energy-PSUM banks
            # so PE never waits for the stream slots to clear; the stream
            # groups close out and store immediately. Smallest tiles last.
            full_group(1, 512, 512, pool=ps_big, tag="engA")
            full_group(0, 1280, 512, pool=ps_big, tag="engB")

            for gi, (oc, j0) in enumerate(STREAM_GROUPS):
                nc.tensor.matmul(
                    st_tiles[gi][:],
                    vt[:, NI - 1, oc * P:(oc + 1) * P],
                    expA[:, NI - 1, j0:j0 + 512],
                    start=False, stop=True)
                emit_out(oc, j0, 512, st_tiles[gi][:])

            full_group(1, 1280, 512)
            full_group(0, 1792, 512)
            full_group(1, 1792, 512, pool=ps_st, tag="st")
            full_group(0, 1024, 256)
            full_group(1, 1024, 256)

    nc.compile()
    return nc


_NC_CACHE = []


def _get_nc():
    if not _NC_CACHE:
        _NC_CACHE.append(_build_nc())
    return _NC_CACHE[0]


def _prep_inputs(x, query_weight, query_bias, key_weight, key_bias,
                 value_weight, value_bias, gamma_weight, gamma_bias):
    bf16 = ml_dtypes.bfloat16
    x = np.asarray(x, np.float32).reshape(B, C, N)
    qw = np.asarray(query_weight, np.float32)[:, :, 0, 0]   # (64, 256)
    kw = np.asarray(key_weight, np.float32)[:, :, 0, 0]     # (64, 256)
    vw = np.asarray(value_weight, np.float32)[:, :, 0, 0]   # (256, 256)
    gw = np.asarray(gamma_weight, np.float32)[:, :, 0, 0]   # (256, 256)

    # wqk[p, c*128+m] = W_cat^T[c*128+p, m]  (W_cat = [Wq; Wk], (128, 256))
    wcat_t = np.concatenate([qw, kw], axis=0).T              # (256, 128)
    wqk = np.ascontiguousarray(
        wcat_t.reshape(2, P, P).transpose(1, 0, 2).reshape(P, 2 * P))

    # the gamma 1x1 conv folds into the V projection:
    #   out = Wg @ (VS^T E) + bg = ((Wv^T Wg^T-projected X)^T-scaled E) + bg
    # so the device projects x with W_comb = (Wg @ Wv)^T and the value bias
    # becomes bvg = Wg @ bv.
    w_comb = (gw @ vw).T                                    # (c_in, o)
    wrest = np.ascontiguousarray(
        w_comb.reshape(2, P, C).transpose(1, 0, 2).reshape(P, 2 * C))
    bvg = gw @ np.asarray(value_bias, np.float32)

    fblob = np.zeros((P, C + 4), np.float32)
    fblob[0:CK, 0] = np.asarray(query_bias, np.float32)
    fblob[0:CK, 1] = np.asarray(key_bias, np.float32)
    fblob[:, 2:4] = np.asarray(gamma_bias, np.float32).reshape(2, P).T
    fblob[:, 4:C + 4] = bvg[None, :]

    base = {
        "wqk": wqk.astype(bf16),
        "wrest": wrest.astype(bf16),
        "fblob": fblob,
    }
    in_maps = []
    for b in range(B):
        m = dict(base)
        m["x"] = x[b].reshape(2, P, N).astype(bf16)
        in_maps.append(m)
    return in_maps


def kernel(x, query_weight, query_bias, key_weight, key_bias,
           value_weight, value_bias, gamma_weight, gamma_bias, k):
    assert int(k) == C // CK
    in_maps = _prep_inputs(x, query_weight, query_bias, key_weight, key_bias,
                           value_weight, value_bias, gamma_weight, gamma_bias)
    nc = _get_nc()
    res = run_bass_kernel_spmd(nc, in_maps, core_ids=list(range(NCORES)))

    out = np.empty((B, C, H, W), np.float32)
    for b in range(B):
        out[b] = res.results[b]["out"].reshape(C, H, W)
    return out
